# revision 41
# baseline (speedup 1.0000x reference)
"""IoU loss kernel for Trainium2, data-parallel over 8 NeuronCores.

Math (per box, columns = x-center, y-center, half-size s):
    w = relu(min(x+s, x'+s') - max(x-s, x'-s'))
      = relu((s+s') - max(|x-x'|, |s-s'|))          # S - max identity
    h likewise with y.
    overlap = w*h
    union   = 4s^2 + 4s'^2 - overlap = 2(S^2 + D^2) - overlap,
              S = s+s', D = s-s'
    iou     = overlap / (union + 1e-7)
    loss    = -sum(log(iou + 1e-7));  iou_sum = sum(iou)

Every op here is validated against neuronxcc codegen on hardware (the
fancier AluOps - abs_max, pow - and Pool scalar_tensor_tensor are
rejected by CoreV3Convert, so they are not used).

Scaling: pre-division quantities are carried x1024 so everything stays
fp16-representable: w_ = relu(wr) * (-32), h_ = relu(hr) * 32, so
ovneg = w_*h_ = -1024*overlap.  t1 = (qsum + eps/2) * 2048, so
u0 = t1 + ovneg = 1024*(union+eps) is a plain 2x tensor_tensor ADD
(a scalar_tensor_tensor here would run in 1x mode).  r'' = Exp(-Ln(u0))
= 1/(1024*(union+eps)) <= ~1e4 fits fp16, and
iou = (ovneg * -1) * r'' exactly via the fused-accum scalar_tensor_tensor.

Engine split per chunk (all four sequencers balanced against the DMA
queue-hold floor):
  Pool  : dx, dy, S, D (fp32 strided channel extraction -> fp16),
          qS = S*S, qD = D*D, qsum = qS+qD; ovneg joins on odd chunks
  DVE   : |dx|,|dy|,|D| via uint16 AND 0x7FFF (4x), mw/mh maxes,
          wr/hr subs, relu-scale (dual-op tensor_scalar, 4x),
          ovneg (even chunks), t1 (dual-op tensor_scalar),
          u0 = t1 + ovneg (2x TT),
          iou = (ovneg * -1) * r'' with fused row-sum accum (STT)
  ACT   : lnu = Ln(u0), r'' = Exp(-lnu), li = Ln(iou + eps) with
          accum_out -> loss partial
  DMA   : outputs stream on the SP HWDGE queue, targets mostly on the
          ACT queue (the two queues transfer concurrently); tail targets
          move to SP so the ACT queue can drain its activation backlog.
  Host  : final [128, 2T] x 8 cores partial-sum reduction in float64.

Chunk widths taper at both ends (256/512s around 1024s) so the first
compute starts ~4.5us in and the last chain after the final DMA is short.

Post-compile surgery: the stock act-table pass maps ln to table set 5 and
exp to set 0, inserting a reload per switch (~1283ns each, ~18 per core).
Both live in set 6 (natural_log_exp_and_others), so we rewrite the first
InstLoadActFuncSet to id 6, delete the rest (they carry no sync_info),
and place the single load after the early DMA issues on the ACT queue.
"""

import numpy as np

import concourse.bass as bass
import concourse.mybir as mybir
from concourse import tile
from concourse.bass_utils import run_bass_kernel_spmd

N = 8388608
NCORES = 8
NS = N // NCORES  # 1048576 boxes per core
P = 128
W = 1024          # boxes per partition per big chunk
NW = NS // P      # 8192 boxes per partition total
EPS = 1e-7
SCALE = 1024.0

F32 = mybir.dt.float32
F16 = mybir.dt.float16
U16 = mybir.dt.uint16
Op = mybir.AluOpType
Act = mybir.ActivationFunctionType

# DMA groups: (compute-chunk widths, outputs-queue, targets-queue)
GROUPS = [
    ([256], "sp", "act"),
    ([256], "sp", "act"),
    ([512], "sp", "act"),
    ([1024], "sp", "act"),
    ([1024], "sp", "act"),
    ([1024], "sp", "act"),
    ([1024], "sp", "sp"),
    ([1024], "sp", "act"),
    ([1024], "sp", "sp"),
    ([512], "sp", "sp"),
    ([256], "sp", "sp"),
    ([256], "sp", "sp"),
]
CHUNKS = [w for g in GROUPS for w in g[0]]
assert sum(CHUNKS) == NW
T = len(CHUNKS)

# natural_log_exp_and_others in act_info.json (contains both ln and exp)
ACT_SET_ALL = 6


def _fuse_act_table_loads(nc) -> None:
    """Keep the first InstLoadActFuncSet (retargeted to the superset table),
    delete the rest.  Loads are inserted after semaphore generation and
    carry no sync_info, so deletion cannot break waiters."""
    first = True
    for blk in nc.main_func.blocks:
        keep = []
        load = None
        for ins in blk.instructions:
            if isinstance(ins, mybir.InstLoadActFuncSet):
                assert ins.sync_info is None or (
                    not ins.sync_info.on_wait and not ins.sync_info.on_update
                )
                if first:
                    ins.act_func_set_id = ACT_SET_ALL
                    first = False
                    load = ins
                continue
            keep.append(ins)
        if load is not None:
            # place the single load right before the first activation so it
            # does not delay the ACT queue's early DMA issues
            for i, ins in enumerate(keep):
                if isinstance(ins, mybir.InstActivation):
                    keep.insert(i, load)
                    break
            else:
                keep.insert(0, load)
        blk.instructions[:] = keep


def _build(groups: list | None = None, compile_passes: bool = True) -> bass.Bass:
    from concourse import bacc

    groups = groups if groups is not None else GROUPS
    chunk_ws = [w for g in groups for w in g[0]]
    T_ = len(chunk_ws)
    nw = sum(chunk_ws)
    ns = P * nw
    nc = bacc.Bacc()
    outs_d = nc.dram_tensor("outputs", [ns, 3], F32, kind="ExternalInput")
    tars_d = nc.dram_tensor("targets", [ns, 3], F32, kind="ExternalInput")
    acc_d = nc.dram_tensor("acc", [P, 2 * T_], F32, kind="ExternalOutput")

    dma_q = {"sp": nc.sync, "act": nc.scalar, "pool": nc.gpsimd}
    GW = max(sum(g[0]) for g in groups)  # widest DMA group

    with tile.TileContext(nc) as tc:
        with tc.tile_pool(name="main", bufs=2) as pool:
            accs = pool.tile([P, 2 * T_], F32, tag="accs", bufs=1)
            eps_t = pool.tile([P, 1], F32, tag="eps", bufs=1)
            nc.vector.memset(eps_t[:, :], EPS)
            # Software-pipelined emission: DMA issues run LAG groups ahead
            # of compute in program order, so an activation waiting on DVE
            # cannot head-of-line-block the next DMA issue on the ACT queue.
            LAG = 2
            off = 0
            t = -1
            staged = {}
            for gi in range(len(groups) + LAG):
                if gi < len(groups):
                    widths, qo, qt = groups[gi]
                    # group covers rows [P*off, P*(off+Wg)): partition-major,
                    # Wg boxes per partition, 3 floats per box; one DMA per
                    # input tensor
                    Wg = sum(widths)
                    o_v = outs_d[P * off : P * (off + Wg), :].rearrange(
                        "(p w) c -> p (w c)", p=P, w=Wg
                    )
                    t_v = tars_d[P * off : P * (off + Wg), :].rearrange(
                        "(p w) c -> p (w c)", p=P, w=Wg
                    )
                    off += Wg
                    rawO = pool.tile([P, 3 * GW], F32, tag="rawO", bufs=4)
                    rawT = pool.tile([P, 3 * GW], F32, tag="rawT", bufs=4)
                    dma_q[qo].dma_start(out=rawO[:, : 3 * Wg], in_=o_v)
                    dma_q[qt].dma_start(out=rawT[:, : 3 * Wg], in_=t_v)
                    staged[gi] = (widths, rawO, rawT)
                ci = gi - LAG
                if ci < 0:
                    continue
                widths, rawO, rawT = staged.pop(ci)
                goff = 0
                for Wc in widths:
                    t += 1
                    o3 = rawO[:, 3 * goff : 3 * (goff + Wc)].rearrange(
                        "p (w c) -> p w c", c=3
                    )
                    t3 = rawT[:, 3 * goff : 3 * (goff + Wc)].rearrange(
                        "p (w c) -> p w c", c=3
                    )
                    goff += Wc
                    x1, y1, s1 = o3[:, :, 0], o3[:, :, 1], o3[:, :, 2]
                    x2, y2, s2 = t3[:, :, 0], t3[:, :, 1], t3[:, :, 2]

                    # Pool: the 4 strided fp32 channel extractions (ordered so
                    # mw's inputs dx, D land first) + squares + their sum
                    dx = pool.tile([P, W], F16, tag="dx", name="dx")[:, :Wc]
                    nc.gpsimd.tensor_tensor(dx, x1, x2, Op.subtract)
                    D = pool.tile([P, W], F16, tag="D", name="D")[:, :Wc]
                    nc.gpsimd.tensor_tensor(D, s1, s2, Op.subtract)
                    dy = pool.tile([P, W], F16, tag="dy", name="dy")[:, :Wc]
                    nc.gpsimd.tensor_tensor(dy, y1, y2, Op.subtract)
                    S = pool.tile([P, W], F16, tag="S", name="S")[:, :Wc]
                    nc.gpsimd.tensor_tensor(S, s1, s2, Op.add)
                    qS = pool.tile([P, W], F16, tag="qS", name="qS")[:, :Wc]
                    nc.gpsimd.tensor_tensor(qS, S, S, Op.mult)
                    qD = pool.tile([P, W], F16, tag="qD", name="qD")[:, :Wc]
                    nc.gpsimd.tensor_tensor(qD, D, D, Op.mult)
                    qs = pool.tile([P, W], F16, tag="qs", name="qs")[:, :Wc]
                    nc.gpsimd.tensor_tensor(qs, qS, qD, Op.add)

                    # |dx|, |dy|, |D| on DVE: clear the fp16 sign bit (4x)
                    adx = pool.tile([P, W], F16, tag="adx", name="adx")[:, :Wc]
                    nc.vector.tensor_scalar(
                        adx.bitcast(U16), dx.bitcast(U16), 0x7FFF, None,
                        Op.bitwise_and,
                    )
                    ady = pool.tile([P, W], F16, tag="ady", name="ady")[:, :Wc]
                    nc.vector.tensor_scalar(
                        ady.bitcast(U16), dy.bitcast(U16), 0x7FFF, None,
                        Op.bitwise_and,
                    )
                    aD = pool.tile([P, W], F16, tag="aD", name="aD")[:, :Wc]
                    nc.vector.tensor_scalar(
                        aD.bitcast(U16), D.bitcast(U16), 0x7FFF, None,
                        Op.bitwise_and,
                    )

                    mw = pool.tile([P, W], F16, tag="mw", name="mw")[:, :Wc]
                    nc.vector.tensor_tensor(mw, adx, aD, Op.max)
                    mh = pool.tile([P, W], F16, tag="mh", name="mh")[:, :Wc]
                    nc.vector.tensor_tensor(mh, ady, aD, Op.max)

                    wr = pool.tile([P, W], F16, tag="wr", name="wr")[:, :Wc]
                    nc.vector.tensor_sub(wr, S, mw)
                    hr = pool.tile([P, W], F16, tag="hr", name="hr")[:, :Wc]
                    nc.vector.tensor_sub(hr, S, mh)

                    # w_ = relu(wr) * 32, h_ = relu(hr) * 32
                    w_ = pool.tile([P, W], F16, tag="w_", name="w_")[:, :Wc]
                    nc.vector.tensor_scalar(w_, wr, 0.0, 32.0, Op.max, Op.mult)
                    h_ = pool.tile([P, W], F16, tag="h_", name="h_")[:, :Wc]
                    nc.vector.tensor_scalar(h_, hr, 0.0, 32.0, Op.max, Op.mult)

                    # ovs = 1024 * overlap; alternates engines to balance
                    # steady-state load
                    ovs = pool.tile([P, W], F16, tag="ovs", name="ovs")[:, :Wc]
                    if t % 2 == 0:
                        nc.vector.tensor_mul(ovs, w_, h_)
                    else:
                        nc.gpsimd.tensor_tensor(ovs, w_, h_, Op.mult)

                    # t1 = (qsum + eps/2) * 2048
                    t1 = pool.tile([P, W], F16, tag="t1", name="t1")[:, :Wc]
                    nc.vector.tensor_scalar(
                        t1, qs, EPS / 2.0, 2.0 * SCALE, Op.add, Op.mult
                    )
                    # u0 = t1 - ovs = 1024*(union+eps)   (2x TT)
                    u0 = pool.tile([P, W], F16, tag="u0", name="u0")[:, :Wc]
                    nc.vector.tensor_sub(u0, t1, ovs)

                    # r'' = 1/u0 via ACT: Exp(-Ln(u0)); <= ~1e4 so fp16 holds
                    lnu = pool.tile([P, W], F32, tag="lnu", name="lnu")[:, :Wc]
                    nc.scalar.activation(lnu, u0, Act.Ln)
                    r = pool.tile([P, W], F16, tag="r", name="r")[:, :Wc]
                    nc.scalar.activation(r, lnu, Act.Exp, scale=-1.0)

                    # iou = ovs * r'' (exact), fused per-chunk row-sum.
                    # accs layout is interleaved [iou_0, loss_0, iou_1, ...]
                    # so chunk t's two accumulators are the contiguous pair
                    # at column 2t, and all-but-the-last can store early.
                    iou = pool.tile([P, W], F16, tag="iou", name="iou")[:, :Wc]
                    nc.vector.scalar_tensor_tensor(
                        iou, ovs, 1.0, r, Op.mult, Op.mult,
                        accum_out=accs[:, 2 * t : 2 * t + 1],
                    )

                    # loss partial: sum of Ln(iou + eps) via accumulate
                    li = pool.tile([P, W], F16, tag="li", name="li")[:, :Wc]
                    nc.scalar.activation(
                        li,
                        iou,
                        Act.Ln,
                        bias=eps_t[:, 0:1],
                        accum_out=accs[:, 2 * t + 1 : 2 * t + 2],
                    )
                    if t == T_ - 2:
                        # all chunks but the last are done: stream their
                        # accumulators out now so the final store is tiny
                        nc.sync.dma_start(
                            out=acc_d[:, : 2 * (T_ - 1)],
                            in_=accs[:, : 2 * (T_ - 1)],
                        )

            nc.sync.dma_start(
                out=acc_d[:, 2 * (T_ - 1) :], in_=accs[:, 2 * (T_ - 1) :]
            )

    if compile_passes:
        nc.compile()
        _fuse_act_table_loads(nc)
    return nc


_NC_CACHE: list[bass.Bass] = []


def _get_nc() -> bass.Bass:
    if not _NC_CACHE:
        _NC_CACHE.append(_build())
    return _NC_CACHE[0]


def _host_reduce(accs: list) -> tuple:
    iou_sum = 0.0
    loss = 0.0
    for a in accs:
        a = np.asarray(a, dtype=np.float64)
        iou_sum += a[:, 0::2].sum()
        loss += a[:, 1::2].sum()
    return -loss, iou_sum


def _run(inputs: dict, trace: bool = False, trace_kwargs: dict | None = None):
    outputs = np.ascontiguousarray(np.asarray(inputs["outputs"], dtype=np.float32))
    targets = np.ascontiguousarray(np.asarray(inputs["targets"], dtype=np.float32))
    assert outputs.shape == (N, 3) and targets.shape == (N, 3)

    nc = _get_nc()
    in_maps = [
        {
            "outputs": outputs[c * NS : (c + 1) * NS],
            "targets": targets[c * NS : (c + 1) * NS],
        }
        for c in range(NCORES)
    ]
    kw = {}
    if trace:
        kw["trace"] = True
        if trace_kwargs:
            kw["trace_kwargs"] = trace_kwargs
    res = run_bass_kernel_spmd(nc, in_maps, list(range(NCORES)), **kw)

    loss, iou_sum = _host_reduce([res.results[c]["acc"] for c in range(NCORES)])
    return (np.float32(loss), np.float32(iou_sum)), res


def kernel(**inputs) -> tuple:
    (loss, iou_sum), _ = _run(inputs)
    return (loss, iou_sum)


# revision 42
# speedup vs baseline: 417064.5849x; 417064.5849x over previous
"""IoU loss kernel for Trainium2, data-parallel over 8 NeuronCores.

Math (per box, columns = x-center, y-center, half-size s):
    w = relu(min(x+s, x'+s') - max(x-s, x'-s'))
      = relu((s+s') - max(|x-x'|, |s-s'|))          # S - max identity
    h likewise with y.
    overlap = w*h
    union   = 4s^2 + 4s'^2 - overlap = 2(S^2 + D^2) - overlap,
              S = s+s', D = s-s'
    iou     = overlap / (union + 1e-7)
    loss    = -sum(log(iou + 1e-7));  iou_sum = sum(iou)

Every op here is validated against neuronxcc codegen on hardware (the
fancier AluOps - abs_max, pow - and Pool scalar_tensor_tensor are
rejected by CoreV3Convert, so they are not used).

Scaling: pre-division quantities are carried x1024 so everything stays
fp16-representable: w_ = relu(wr) * 32, h_ = relu(hr) * 32, so
ovs = w_*h_ = 1024*overlap.  t1 = (qsum + eps/2) * 2048, so
u0 = t1 - ovs = 1024*(union+eps) is a plain 2x tensor_tensor SUB
(a scalar_tensor_tensor here would run in 1x mode).  r'' = Exp(-Ln(u0))
= 1/(1024*(union+eps)) <= ~1e4 fits fp16, and iou = ovs * r'' exactly
via the fused-accum scalar_tensor_tensor.

Engine split per chunk (all four sequencers balanced against the DMA
queue-hold floor):
  Pool  : dx, dy, S, D (fp32 strided channel extraction -> fp16),
          qS = S*S, qD = D*D, qsum = qS+qD; ovs joins on odd chunks
  DVE   : |dx|,|dy|,|D| via uint16 AND 0x7FFF (4x), mw/mh maxes,
          wr/hr subs, relu-scale (dual-op tensor_scalar, 4x),
          ovs (even chunks), t1 (dual-op tensor_scalar),
          u0 = t1 - ovs (2x TT),
          iou = ovs * r'' with fused row-sum accum (STT)
  ACT   : lnu = Ln(u0), r'' = Exp(-lnu), li = Ln(iou + eps) with
          accum_out -> loss partial
  DMA   : outputs stream on the SP HWDGE queue, targets mostly on the
          ACT queue (the two queues transfer concurrently); tail targets
          move to SP so the ACT queue can drain its activation backlog.
          The accumulators for all-but-the-last chunk stream out early so
          the final store is a tiny 2-column DMA.
  Host  : final [128, 2T] x 8 cores partial-sum reduction in float64
          (accs layout interleaves iou/loss pairs per chunk).

Chunk widths taper at both ends (256/512s around 1024s) so the first
compute starts ~4.5us in and the last chain after the final DMA is short.

Post-compile surgery: the stock act-table pass maps ln to table set 5 and
exp to set 0, inserting a reload per switch (~1283ns each, ~18 per core).
Both live in set 6 (natural_log_exp_and_others), so we rewrite the first
InstLoadActFuncSet to id 6, delete the rest (they carry no sync_info),
and place the single load after the early DMA issues on the ACT queue.
"""

import numpy as np

import concourse.bass as bass
import concourse.mybir as mybir
from concourse import tile
from concourse.bass_utils import run_bass_kernel_spmd

N = 8388608
NCORES = 8
NS = N // NCORES  # 1048576 boxes per core
P = 128
W = 1024          # boxes per partition per big chunk
NW = NS // P      # 8192 boxes per partition total
EPS = 1e-7
SCALE = 1024.0

F32 = mybir.dt.float32
F16 = mybir.dt.float16
U16 = mybir.dt.uint16
Op = mybir.AluOpType
Act = mybir.ActivationFunctionType

# DMA groups: (compute-chunk widths, outputs-queue, targets-queue)
GROUPS = [
    ([256], "sp", "act"),
    ([256], "sp", "act"),
    ([512], "sp", "act"),
    ([1024], "sp", "act"),
    ([1024], "sp", "act"),
    ([1024], "sp", "act"),
    ([1024], "sp", "sp"),
    ([1024], "sp", "act"),
    ([1024], "sp", "sp"),
    ([512], "sp", "sp"),
    ([256], "sp", "sp"),
    ([256], "sp", "sp"),
]
CHUNKS = [w for g in GROUPS for w in g[0]]
assert sum(CHUNKS) == NW
T = len(CHUNKS)

# natural_log_exp_and_others in act_info.json (contains both ln and exp)
ACT_SET_ALL = 6


def _fuse_act_table_loads(nc) -> None:
    """Keep the first InstLoadActFuncSet (retargeted to the superset table),
    delete the rest.  Loads are inserted after semaphore generation and
    carry no sync_info, so deletion cannot break waiters."""
    first = True
    for blk in nc.main_func.blocks:
        keep = []
        load = None
        for ins in blk.instructions:
            if isinstance(ins, mybir.InstLoadActFuncSet):
                assert ins.sync_info is None or (
                    not ins.sync_info.on_wait and not ins.sync_info.on_update
                )
                if first:
                    ins.act_func_set_id = ACT_SET_ALL
                    first = False
                    load = ins
                continue
            keep.append(ins)
        if load is not None:
            # place the single load right before the first activation so it
            # does not delay the ACT queue's early DMA issues
            for i, ins in enumerate(keep):
                if isinstance(ins, mybir.InstActivation):
                    keep.insert(i, load)
                    break
            else:
                keep.insert(0, load)
        blk.instructions[:] = keep


def _build(groups: list | None = None, compile_passes: bool = True) -> bass.Bass:
    from concourse import bacc

    groups = groups if groups is not None else GROUPS
    chunk_ws = [w for g in groups for w in g[0]]
    T_ = len(chunk_ws)
    nw = sum(chunk_ws)
    ns = P * nw
    nc = bacc.Bacc()
    outs_d = nc.dram_tensor("outputs", [ns, 3], F32, kind="ExternalInput")
    tars_d = nc.dram_tensor("targets", [ns, 3], F32, kind="ExternalInput")
    acc_d = nc.dram_tensor("acc", [P, 2 * T_], F32, kind="ExternalOutput")

    dma_q = {"sp": nc.sync, "act": nc.scalar, "pool": nc.gpsimd}
    GW = max(sum(g[0]) for g in groups)  # widest DMA group

    with tile.TileContext(nc) as tc:
        with tc.tile_pool(name="main", bufs=2) as pool:
            accs = pool.tile([P, 2 * T_], F32, tag="accs", bufs=1)
            eps_t = pool.tile([P, 1], F32, tag="eps", bufs=1)
            nc.vector.memset(eps_t[:, :], EPS)
            # Software-pipelined emission: DMA issues run LAG groups ahead
            # of compute in program order, so an activation waiting on DVE
            # cannot head-of-line-block the next DMA issue on the ACT queue.
            LAG = 2
            off = 0
            t = -1
            staged = {}
            for gi in range(len(groups) + LAG):
                if gi < len(groups):
                    widths, qo, qt = groups[gi]
                    # group covers rows [P*off, P*(off+Wg)): partition-major,
                    # Wg boxes per partition, 3 floats per box; one DMA per
                    # input tensor
                    Wg = sum(widths)
                    o_v = outs_d[P * off : P * (off + Wg), :].rearrange(
                        "(p w) c -> p (w c)", p=P, w=Wg
                    )
                    t_v = tars_d[P * off : P * (off + Wg), :].rearrange(
                        "(p w) c -> p (w c)", p=P, w=Wg
                    )
                    off += Wg
                    rawO = pool.tile([P, 3 * GW], F32, tag="rawO", bufs=4)
                    rawT = pool.tile([P, 3 * GW], F32, tag="rawT", bufs=4)
                    dma_q[qo].dma_start(out=rawO[:, : 3 * Wg], in_=o_v)
                    dma_q[qt].dma_start(out=rawT[:, : 3 * Wg], in_=t_v)
                    staged[gi] = (widths, rawO, rawT)
                ci = gi - LAG
                if ci < 0:
                    continue
                widths, rawO, rawT = staged.pop(ci)
                goff = 0
                for Wc in widths:
                    t += 1
                    o3 = rawO[:, 3 * goff : 3 * (goff + Wc)].rearrange(
                        "p (w c) -> p w c", c=3
                    )
                    t3 = rawT[:, 3 * goff : 3 * (goff + Wc)].rearrange(
                        "p (w c) -> p w c", c=3
                    )
                    goff += Wc
                    x1, y1, s1 = o3[:, :, 0], o3[:, :, 1], o3[:, :, 2]
                    x2, y2, s2 = t3[:, :, 0], t3[:, :, 1], t3[:, :, 2]

                    # Pool: the 4 strided fp32 channel extractions (ordered so
                    # mw's inputs dx, D land first) + squares + their sum
                    dx = pool.tile([P, W], F16, tag="dx", name="dx")[:, :Wc]
                    nc.gpsimd.tensor_tensor(dx, x1, x2, Op.subtract)
                    D = pool.tile([P, W], F16, tag="D", name="D")[:, :Wc]
                    nc.gpsimd.tensor_tensor(D, s1, s2, Op.subtract)
                    dy = pool.tile([P, W], F16, tag="dy", name="dy")[:, :Wc]
                    nc.gpsimd.tensor_tensor(dy, y1, y2, Op.subtract)
                    S = pool.tile([P, W], F16, tag="S", name="S")[:, :Wc]
                    nc.gpsimd.tensor_tensor(S, s1, s2, Op.add)
                    qS = pool.tile([P, W], F16, tag="qS", name="qS")[:, :Wc]
                    nc.gpsimd.tensor_tensor(qS, S, S, Op.mult)
                    qD = pool.tile([P, W], F16, tag="qD", name="qD")[:, :Wc]
                    nc.gpsimd.tensor_tensor(qD, D, D, Op.mult)
                    qs = pool.tile([P, W], F16, tag="qs", name="qs")[:, :Wc]
                    nc.gpsimd.tensor_tensor(qs, qS, qD, Op.add)

                    # |dx|, |dy|, |D| on DVE: clear the fp16 sign bit (4x)
                    adx = pool.tile([P, W], F16, tag="adx", name="adx")[:, :Wc]
                    nc.vector.tensor_scalar(
                        adx.bitcast(U16), dx.bitcast(U16), 0x7FFF, None,
                        Op.bitwise_and,
                    )
                    ady = pool.tile([P, W], F16, tag="ady", name="ady")[:, :Wc]
                    nc.vector.tensor_scalar(
                        ady.bitcast(U16), dy.bitcast(U16), 0x7FFF, None,
                        Op.bitwise_and,
                    )
                    aD = pool.tile([P, W], F16, tag="aD", name="aD")[:, :Wc]
                    nc.vector.tensor_scalar(
                        aD.bitcast(U16), D.bitcast(U16), 0x7FFF, None,
                        Op.bitwise_and,
                    )

                    mw = pool.tile([P, W], F16, tag="mw", name="mw")[:, :Wc]
                    nc.vector.tensor_tensor(mw, adx, aD, Op.max)
                    mh = pool.tile([P, W], F16, tag="mh", name="mh")[:, :Wc]
                    nc.vector.tensor_tensor(mh, ady, aD, Op.max)

                    wr = pool.tile([P, W], F16, tag="wr", name="wr")[:, :Wc]
                    nc.vector.tensor_sub(wr, S, mw)
                    hr = pool.tile([P, W], F16, tag="hr", name="hr")[:, :Wc]
                    nc.vector.tensor_sub(hr, S, mh)

                    # w_ = relu(wr) * 32, h_ = relu(hr) * 32
                    w_ = pool.tile([P, W], F16, tag="w_", name="w_")[:, :Wc]
                    nc.vector.tensor_scalar(w_, wr, 0.0, 32.0, Op.max, Op.mult)
                    h_ = pool.tile([P, W], F16, tag="h_", name="h_")[:, :Wc]
                    nc.vector.tensor_scalar(h_, hr, 0.0, 32.0, Op.max, Op.mult)

                    # ovs = 1024 * overlap; alternates engines to balance
                    # steady-state load
                    ovs = pool.tile([P, W], F16, tag="ovs", name="ovs")[:, :Wc]
                    if t % 2 == 0:
                        nc.vector.tensor_mul(ovs, w_, h_)
                    else:
                        nc.gpsimd.tensor_tensor(ovs, w_, h_, Op.mult)

                    # t1 = (qsum + eps/2) * 2048
                    t1 = pool.tile([P, W], F16, tag="t1", name="t1")[:, :Wc]
                    nc.vector.tensor_scalar(
                        t1, qs, EPS / 2.0, 2.0 * SCALE, Op.add, Op.mult
                    )
                    # u0 = t1 - ovs = 1024*(union+eps)   (2x TT)
                    u0 = pool.tile([P, W], F16, tag="u0", name="u0")[:, :Wc]
                    nc.vector.tensor_sub(u0, t1, ovs)

                    # r'' = 1/u0 via ACT: Exp(-Ln(u0)); <= ~1e4 so fp16 holds
                    lnu = pool.tile([P, W], F32, tag="lnu", name="lnu")[:, :Wc]
                    nc.scalar.activation(lnu, u0, Act.Ln)
                    r = pool.tile([P, W], F16, tag="r", name="r")[:, :Wc]
                    nc.scalar.activation(r, lnu, Act.Exp, scale=-1.0)

                    # iou = ovs * r'' (exact), fused per-chunk row-sum.
                    # accs layout is interleaved [iou_0, loss_0, iou_1, ...]
                    # so chunk t's two accumulators are the contiguous pair
                    # at column 2t, and all-but-the-last can store early.
                    iou = pool.tile([P, W], F16, tag="iou", name="iou")[:, :Wc]
                    nc.vector.scalar_tensor_tensor(
                        iou, ovs, 1.0, r, Op.mult, Op.mult,
                        accum_out=accs[:, 2 * t : 2 * t + 1],
                    )

                    # loss partial: sum of Ln(iou + eps) via accumulate
                    li = pool.tile([P, W], F16, tag="li", name="li")[:, :Wc]
                    nc.scalar.activation(
                        li,
                        iou,
                        Act.Ln,
                        bias=eps_t[:, 0:1],
                        accum_out=accs[:, 2 * t + 1 : 2 * t + 2],
                    )
                    if t == T_ - 2:
                        # all chunks but the last are done: stream their
                        # accumulators out now so the final store is tiny
                        nc.sync.dma_start(
                            out=acc_d[:, : 2 * (T_ - 1)],
                            in_=accs[:, : 2 * (T_ - 1)],
                        )

            nc.sync.dma_start(
                out=acc_d[:, 2 * (T_ - 1) :], in_=accs[:, 2 * (T_ - 1) :]
            )

    if compile_passes:
        nc.compile()
        _fuse_act_table_loads(nc)
    return nc


_NC_CACHE: list[bass.Bass] = []


def _get_nc() -> bass.Bass:
    if not _NC_CACHE:
        _NC_CACHE.append(_build())
    return _NC_CACHE[0]


def _host_reduce(accs: list) -> tuple:
    iou_sum = 0.0
    loss = 0.0
    for a in accs:
        a = np.asarray(a, dtype=np.float64)
        iou_sum += a[:, 0::2].sum()
        loss += a[:, 1::2].sum()
    return -loss, iou_sum


def _run(inputs: dict, trace: bool = False, trace_kwargs: dict | None = None):
    outputs = np.ascontiguousarray(np.asarray(inputs["outputs"], dtype=np.float32))
    targets = np.ascontiguousarray(np.asarray(inputs["targets"], dtype=np.float32))
    assert outputs.shape == (N, 3) and targets.shape == (N, 3)

    nc = _get_nc()
    in_maps = [
        {
            "outputs": outputs[c * NS : (c + 1) * NS],
            "targets": targets[c * NS : (c + 1) * NS],
        }
        for c in range(NCORES)
    ]
    kw = {}
    if trace:
        kw["trace"] = True
        if trace_kwargs:
            kw["trace_kwargs"] = trace_kwargs
    res = run_bass_kernel_spmd(nc, in_maps, list(range(NCORES)), **kw)

    loss, iou_sum = _host_reduce([res.results[c]["acc"] for c in range(NCORES)])
    return (np.float32(loss), np.float32(iou_sum)), res


def kernel(**inputs) -> tuple:
    (loss, iou_sum), _ = _run(inputs)
    return (loss, iou_sum)
